# revision 2
# baseline (speedup 1.0000x reference)
"""Trainium2 Bass kernel for nn_Attention_39015482916872 (v3).

Multi-head attention (B=2, N=2048, C=1024, H=16, D=64) with RoPE,
tensor-parallel over (batch, heads) across 8 NeuronCores: core c handles
batch c//4 and heads 4*(c%4)..4*(c%4)+3. Host sums the 4 partial
projections per batch (Megatron column-parallel w_proj) and adds b_proj.

v3 design (cost-model-driven):
 - x is cast to bf16 AND transposed to [C, N] on the host; device does
   plain DMA loads into SBUF.
 - The kernel is emitted as 128 "exp slots" (one per softmax activation,
   the ACT-engine pacer). Per slot: scheduled PE filler units, the two
   score matmuls (MM1), the exp. PV matmuls (MM2, free size 65->64 via a
   separate denominator accumulator) lag their exp by a per-block LAG,
   bounded by a 32-deep es pool; projection / V / output-projection
   units are placed greedily by deadline into the leftover PE budget.
 - PSUM (8 banks): ps pool 2x2 banks, po_all [128,2,8,64] manual
   double-buffer 2 banks, den_all [128,2,8] 1 bank, pmm_all [128,2,256]
   manual double-buffer 1 bank (all projection units have free size 256).
 - Attention output a^T [q, v] -> DRAM -> XBAR transpose -> [v, q] for
   the output projection.
"""

import sys
from contextlib import ExitStack

import numpy as np

if "/opt/trn_rl_repo" not in sys.path:
    sys.path.insert(0, "/opt/trn_rl_repo")
try:
    import concourse.bass as bass
except ImportError:
    sys.path.insert(0, "/root/.axon_site/_ro/trn_rl_repo")
    import concourse.bass as bass
import concourse.tile as tile
from concourse import bacc, mybir
from concourse.bass_utils import run_bass_kernel_spmd

F32 = mybir.dt.float32
BF16 = mybir.dt.bfloat16
AF = mybir.ActivationFunctionType

B, N, C, H, D = 2, 2048, 1024, 16, 64
SCHED = {
    "LAG": [24, 23, 22, 21, 20, 19, 18, 17],
    "QDL": {(0, 1): 13, (0, 2): 26, (0, 3): 42, (1, 0): 57,
            (1, 1): 73, (1, 2): 89, (1, 3): 105},
    "KDL1": 60,
    "VDL_OFF": -2,
}
N_CORES = 8
CORES_PER_BATCH = N_CORES // B          # 4
HPC = H // CORES_PER_BATCH              # 4 heads per core


def build_attn_kernel(nc, tc, ctx, N=2048, C=1024, HPC=4, D=64, NQ_BLK=512, scale=None, debug=False):
    P = 128
    KC = C // P                          # 8 contraction chunks
    VF = HPC * D                         # 256
    NB = N // NQ_BLK                     # 4 q blocks
    NKC = N // P                         # 16 key chunks
    NCH = HPC // 2                       # 2 head pairs
    QS = NQ_BLK // P                     # 4 q sub-blocks per q block
    NBLK = NCH * NB                      # 8 attention blocks
    NSLOT = NBLK * NKC                   # 128 exp slots
    if scale is None:
        scale = D ** -0.5

    xT = nc.dram_tensor("xT", [8, C, N // 8], BF16, kind="ExternalInput").ap()
    wqkT = nc.dram_tensor("wqkT", [4, C, 2 * HPC * D // 4], BF16, kind="ExternalInput").ap()
    wvT = nc.dram_tensor("wvT", [C, VF], BF16, kind="ExternalInput").ap()
    wpT = nc.dram_tensor("wpT", [VF, C], BF16, kind="ExternalInput").ap()
    cosT = nc.dram_tensor("cosT", [D // 2, N], BF16, kind="ExternalInput").ap()
    sinT = nc.dram_tensor("sinT", [D, N], BF16, kind="ExternalInput").ap()
    ident = nc.dram_tensor("ident", [P, P], BF16, kind="ExternalInput").ap()
    y = nc.dram_tensor("y", [N, C], F32, kind="ExternalOutput").ap()
    adr = nc.dram_tensor("adr_internal", [VF // P, N, P], BF16,
                         kind=("ExternalOutput" if debug else "Internal")).ap()

    persist = ctx.enter_context(tc.tile_pool(name="persist", bufs=1))
    psum_s = ctx.enter_context(tc.tile_pool(name="psum_s", bufs=2, space="PSUM"))
    psum_fix = ctx.enter_context(tc.tile_pool(name="psum_fix", bufs=1, space="PSUM"))
    rope_tmp = ctx.enter_context(tc.tile_pool(name="rope_tmp", bufs=6))
    exp_pool = ctx.enter_context(tc.tile_pool(name="exp_pool", bufs=(36 if debug else 40)))
    norm_pool = ctx.enter_context(tc.tile_pool(name="norm_pool", bufs=4))
    y_pool = ctx.enter_context(tc.tile_pool(name="y_pool", bufs=3))

    xT_sb = persist.tile([P, KC, N], BF16, tag="xTsb")
    wqkT_sb = persist.tile([P, KC, 2 * HPC * D], BF16, tag="wqk")
    wvT_sb = persist.tile([P, KC, VF], BF16, tag="wv")
    wpT_sb = persist.tile([P, VF // P, C], BF16, tag="wp")
    cos_sb = persist.tile([P, N], BF16, tag="cos")
    sin_sb = persist.tile([P, N], BF16, tag="sin")
    ones_sb = persist.tile([P, 1], BF16, tag="ones")
    ident_sb = persist.tile([P, P], BF16, tag="ident")
    zeros_sb = persist.tile([P, 512], BF16, tag="zeros")
    qt = [[persist.tile([P, NQ_BLK], BF16, name=f"qt{i}_{j}", tag=f"qt{i}_{j}") for j in range(NB)] for i in range(NCH)]
    kt = [[persist.tile([P, NQ_BLK], BF16, name=f"kt{i}_{j}", tag=f"kt{i}_{j}") for j in range(NB)] for i in range(NCH)]
    vt = persist.tile([P, NKC, HPC, D], BF16, tag="vt")
    aTT = persist.tile([P, VF // P, N], BF16, tag="aTT")

    # PSUM plan: a matmul with start=True zeroes its whole 2KB region, so
    # each open accumulation bank gets ONE dummy start per block and all
    # real matmuls accumulate with start=False. po and den are single
    # banks (blocks serialize through them via the norm WAR dep); pmm is
    # a 4-slice ring for the atomic projection units.
    po_all = psum_fix.tile([P, 2 * QS, D], F32, tag="po_all")
    den_all = psum_fix.tile([P, 2 * QS], F32, tag="den_all")
    pmm_all = psum_fix.tile([P, 4, 256], F32, tag="pmm_all")

    # preload the exp activation table while DMAs stream in
    warm = persist.tile([1, 8], F32, tag="actwarm")
    nc.vector.memset(warm[:], 0.0)
    nc.scalar.activation(warm[:], warm[:], AF.Exp, scale=1.0)
    nc.vector.memset(ones_sb[:], 1.0)
    nc.vector.memset(zeros_sb[:], 0.0)

    # --- startup DMAs -----------------------------------------------------
    # ALL on the SP queue: DMA transfers serialize on the engine pool, so
    # queue emission order is transfer priority.
    def wqk_g(g):
        return wqkT[g].rearrange("(kc p) f -> p kc f", p=P)

    def x_chunk(c):
        return xT[c].rearrange("(kc p) n -> p kc n", p=P)

    nc.sync.dma_start(wqkT_sb[:, :, 0:P], wqk_g(0))           # K i0
    nc.sync.dma_start(xT_sb[:, :, 0:256], x_chunk(0))
    nc.sync.dma_start(wqkT_sb[:, :, P:2 * P], wqk_g(1))       # Q i0
    for g in range(P // (D // 2)):
        nc.sync.dma_start(cos_sb[g * (D // 2):(g + 1) * (D // 2), :], cosT[:, :])
    for g in range(P // D):
        nc.sync.dma_start(sin_sb[g * D:(g + 1) * D, :], sinT[:, :])
    nc.sync.dma_start(xT_sb[:, :, 256:512], x_chunk(1))
    nc.sync.dma_start(xT_sb[:, :, 512:768], x_chunk(2))
    nc.sync.dma_start(xT_sb[:, :, 768:1024], x_chunk(3))
    nc.sync.dma_start(wqkT_sb[:, :, 2 * P:3 * P], wqk_g(2))   # K i1
    nc.sync.dma_start(wqkT_sb[:, :, 3 * P:4 * P], wqk_g(3))   # Q i1
    nc.sync.dma_start(wvT_sb[:], wvT.rearrange("(kc p) f -> p kc f", p=P))
    nc.sync.dma_start(xT_sb[:, :, 1024:1280], x_chunk(4))
    nc.sync.dma_start(xT_sb[:, :, 1280:1536], x_chunk(5))
    nc.sync.dma_start(xT_sb[:, :, 1536:1792], x_chunk(6))
    nc.sync.dma_start(xT_sb[:, :, 1792:2048], x_chunk(7))
    nc.sync.dma_start(wpT_sb[:], wpT.rearrange("(vc p) f -> p vc f", p=P))
    nc.sync.dma_start(ident_sb[:], ident)

    def xs(k, n0, w):
        return xT_sb[:, k, n0:n0 + w]

    # --- projection / filler units (each uses one pmm_all slice) ---------
    pmm_ctr = [0]

    def pmm_slice():
        s = pmm_ctr[0] % 4
        pmm_ctr[0] += 1
        return pmm_all[:, s, :]

    def v_chunk(kk):
        pv = pmm_slice()
        for k in range(KC):
            nc.tensor.matmul(
                pv, lhsT=xs(k, kk * P, P), rhs=wvT_sb[:, k, :],
                start=(k == 0), stop=(k == KC - 1),
            )
        nc.vector.tensor_copy(vt[:, kk, :, :], pv.rearrange("p (h d) -> p h d", d=D))

    def rope_half(psum_c, dst, j, half, act_copy=False):
        nb = j * NQ_BLK + half * 256
        cs = cos_sb[:, nb:nb + 256]
        sn = sin_sb[:, nb:nb + 256]
        raw = rope_tmp.tile([P, 256], BF16, tag="raw")
        if act_copy:
            nc.scalar.activation(raw[:], psum_c, AF.Copy)
        else:
            nc.vector.tensor_copy(raw[:], psum_c)
        tA = rope_tmp.tile([P, 256], BF16, tag="tA")
        tB = rope_tmp.tile([P, 256], BF16, tag="tB")
        nc.vector.tensor_mul(tA[:], raw[:], cs)
        # swapped sin product: out rows swap r<->i; +/- folded into sin table
        for g in range(2):
            b0 = 64 * g
            nc.vector.tensor_mul(tB[b0:b0 + 32, :], raw[b0 + 32:b0 + 64, :], sn[b0 + 32:b0 + 64, :])
            nc.vector.tensor_mul(tB[b0 + 32:b0 + 64, :], raw[b0:b0 + 32, :], sn[b0:b0 + 32, :])
        nc.vector.tensor_add(dst[:, nb - j * NQ_BLK:nb - j * NQ_BLK + 256], tA[:], tB[:])

    def qk_unit(qk, i, j, half, act_copy=False):
        """n-half (256 cols) of a Q/K projection block, incl. RoPE."""
        fbase = i * 2 * P + (1 - qk) * P
        pqk = pmm_slice()
        n0 = j * NQ_BLK + half * 256
        for k in range(KC):
            nc.tensor.matmul(
                pqk, lhsT=wqkT_sb[:, k, fbase:fbase + P], rhs=xs(k, n0, 256),
                start=(k == 0), stop=(k == KC - 1),
            )
        dst = (qt if qk == 0 else kt)[i][j]
        rope_half(pqk, dst, j, half, act_copy=act_copy)

    # --- phase 3: output projection --------------------------------------
    def transpose_a(j, vc):
        nc.sync.dma_start_transpose(
            aTT[:, vc, j * NQ_BLK:(j + 1) * NQ_BLK],
            adr[vc, j * NQ_BLK:(j + 1) * NQ_BLK, :],
        )

    OB = 256
    NOB = C // OB
    yt_tiles = {}
    p3_ctr = [0]

    in_tail_flag = [False]

    def tail_py(u):
        # post-exp psum ring: the 4 pmm slices + the freed MM1 score pool
        if u % 3 == 0:
            py_t = psum_s.tile([P, 2, NQ_BLK], F32, name="ps", tag="ps")
            return py_t[:, 0, 0:OB]
        return pmm_slice()

    def phase3_unit(jj, ob):
        u = p3_ctr[0]
        p3_ctr[0] += 1
        in_tail = in_tail_flag[0]
        if in_tail:
            py = tail_py(u)
        else:
            py = pmm_slice()
        for vc in range(VF // P):
            nc.tensor.matmul(
                py, lhsT=aTT[:, vc, jj * P:(jj + 1) * P],
                rhs=wpT_sb[:, vc, ob * OB:(ob + 1) * OB],
                start=(vc == 0), stop=(vc == VF // P - 1),
            )
        if jj not in yt_tiles:
            yt_tiles[jj] = y_pool.tile([P, C], F32, name="yt", tag="yt")
        yt = yt_tiles[jj]
        # GPSIMD cannot read PSUM: DVE in-stream; rotate DVE/ACT in the
        # tail (ACT is idle once the exps are done)
        if in_tail and u % 2 == 0:
            nc.scalar.activation(yt[:, ob * OB:(ob + 1) * OB], py, AF.Copy)
        else:
            nc.vector.tensor_copy(yt[:, ob * OB:(ob + 1) * OB], py)
        if ob == NOB - 1:
            nc.sync.dma_start(y[jj * P:(jj + 1) * P, :], yt[:])
            del yt_tiles[jj]

    # --- phase 2 pieces ----------------------------------------------------
    es_tiles = {}
    debug_es = []

    def mm1_exp(b, kk):
        i, j = b // NB, b % NB
        h0 = 2 * i
        ps = psum_s.tile([P, 2, NQ_BLK], F32, tag="ps")
        kb, kc0 = divmod(kk * P, NQ_BLK)
        for g in range(2):
            hb = 64 * g
            nc.tensor.matmul(
                ps[:, g, :],
                lhsT=kt[i][kb][hb:hb + 64, kc0:kc0 + P],
                rhs=qt[i][j][hb:hb + 64, :],
                start=True, stop=True,
            )
        es = exp_pool.tile([P, 2, NQ_BLK], BF16, tag="es")
        nc.scalar.activation(es[:], ps[:], AF.Exp, scale=float(scale))
        es_tiles[(b, kk)] = es
        if (b, kk) == (0, 0) and not debug_es:
            debug_es.append(es)

    def mm2_batch(b, kk):
        i = b // NB
        h0, h1 = 2 * i, 2 * i + 1
        es = es_tiles.pop((b, kk))
        if kk == 0:
            # open the po and den banks: one start=True zeroing matmul each
            nc.tensor.matmul(po_all[:, :, :], lhsT=ident_sb[:], rhs=zeros_sb[:],
                             start=True, stop=False)
            nc.tensor.matmul(den_all[:, :], lhsT=ident_sb[:], rhs=zeros_sb[:, 0:2 * QS],
                             start=True, stop=False)
        last = (kk == NKC - 1)
        for qs in range(QS):
            for g, h in ((0, h0), (1, h1)):
                idx = qs * 2 + g
                lhsT = es[:, g, qs * P:(qs + 1) * P]
                nc.tensor.matmul(
                    po_all[:, idx, :], lhsT=lhsT, rhs=vt[:, kk, h, :],
                    start=False, stop=(last and idx == 2 * QS - 1),
                )
                nc.tensor.matmul(
                    den_all[:, idx:idx + 1], lhsT=lhsT, rhs=ones_sb[:],
                    start=False, stop=(last and idx == 2 * QS - 1),
                )

    def mm2_idx_unit(b, idx):
        i = b // NB
        qs, g = idx // 2, idx % 2
        h = 2 * i + g
        last = 2 * QS - 1
        for kk in range(NKC):
            lhsT = es_tiles[(b, kk)][:, g, qs * P:(qs + 1) * P]
            nc.tensor.matmul(
                po_all[:, idx, :], lhsT=lhsT, rhs=vt[:, kk, h, :],
                start=(kk == 0), stop=(kk == NKC - 1),
            )
        for kk in range(NKC):
            lhsT = es_tiles[(b, kk)][:, g, qs * P:(qs + 1) * P]
            nc.tensor.matmul(
                den_all[:, idx:idx + 1], lhsT=lhsT, rhs=ones_sb[:],
                start=(kk == 0), stop=(kk == NKC - 1),
            )
        if idx == last:
            for kk in range(NKC):
                es_tiles.pop((b, kk))

    def norm_store(b):
        i, j = b // NB, b % NB
        if debug and b == 0:
            dbg_den = nc.dram_tensor("dbg_den", [P, 2 * QS], F32, kind="ExternalOutput").ap()
            dbg_po = nc.dram_tensor("dbg_po", [P, 2 * QS, D], F32, kind="ExternalOutput").ap()
            dent = persist.tile([P, 2 * QS], F32, tag="dbgden")
            pot = persist.tile([P, 2 * QS, D], F32, tag="dbgpo")
            nc.vector.tensor_copy(dent[:], den_all[:, :])
            nc.vector.tensor_copy(pot[:], po_all[:, :, :])
            nc.sync.dma_start(dbg_den, dent[:])
            nc.sync.dma_start(dbg_po, pot[:])
        recip = norm_pool.tile([P, 2 * QS, 1], F32, tag="recip")
        nc.vector.reciprocal(recip[:], den_all[:, :].rearrange("p (q o) -> p q o", o=1))
        aT = norm_pool.tile([P, 2 * QS, D], BF16, tag="aT")
        for idx in range(2 * QS):
            nc.vector.tensor_scalar_mul(
                aT[:, idx, :], po_all[:, idx, :], recip[:, idx, :])
        if i == 1 and j == NB - 1:
            norm_store.last_aT = aT
            return
        dst = adr[i, j * NQ_BLK:(j + 1) * NQ_BLK, :].rearrange(
            "(qs p) (g d) -> p qs g d", p=P, d=D
        )
        nc.sync.dma_start(dst, aT[:].rearrange("p (qs g) d -> p qs g d", g=2))
        transpose_a(j, i)

    # --- static slot schedule ---------------------------------------------
    MM1_NS, MM2_NS = 426, 250
    slots = [[] for _ in range(NSLOT)]
    tail = []

    def place(s, fn):
        (slots[s] if s < NSLOT else tail).append(fn)

    # MM2 trains + norms through the single po/den banks
    # (LAG[b+1] >= LAG[b]-1 keeps the bank serialization legal).
    LAG = SCHED["LAG"]
    mm2_at = {}
    for b in range(NBLK):
        for kk in range(NKC):
            s = 16 * b + kk + LAG[b]
            mm2_at.setdefault(s, []).append((b, kk))
    for s, lst in mm2_at.items():
        for b, kk in lst:
            place(s, (lambda b=b, kk=kk: mm2_batch(b, kk)))
            if kk == NKC - 1:
                place(s, (lambda b=b: norm_store(b)))

    # greedy filler placement: earliest-deadline-first into accumulated
    # PE-idle budget (carry); deadline-due units force regardless of carry.
    units = []   # (deadline, ready, cost, fn)
    kdl = {1: (0, 1), 2: (2, 5), 3: (4, 8)}   # K(0,j): (ready, dl_h0)
    for j, (ready, dl) in kdl.items():
        for half in range(2):
            units.append((dl + half, ready, 853,
                          (lambda j=j, h=half: qk_unit(1, 0, j, h))))
    qdl = SCHED["QDL"]
    for (i, j), dl in qdl.items():
        for half in range(2):
            units.append((dl + half, 1 if j > 0 or i > 0 else 0, 853,
                          (lambda i=i, j=j, h=half: qk_unit(0, i, j, h))))
    for kb in range(NB):
        for half in range(2):
            units.append((SCHED["KDL1"] + 4 * kb + half, 1, 853,
                          (lambda kb=kb, h=half: qk_unit(1, 1, kb, h))))
    for kk in range(NKC):
        ready = 1 if kk < 8 else (3 if kk < 12 else 5)
        units.append((kk + LAG[0] + SCHED["VDL_OFF"], ready, 853,
                      (lambda kk=kk: v_chunk(kk))))
    for j in range(3):
        ready = 16 * (4 + j) + 15 + LAG[4 + j] + 2
        for u in range(NOB * QS):
            # staggered deadlines force ~1 unit/slot right after readiness
            units.append((min(ready + 2 + u, NSLOT - 1), ready, 240,
                          (lambda j=j, u=u: phase3_unit(j * QS + u // NOB, u % NOB))))

    units.sort(key=lambda t: (t[0], t[1]))
    pending = list(units)
    carry = 0.0
    for s in range(NSLOT):
        carry += 1038 - MM1_NS - MM2_NS * len(mm2_at.get(s, []))
        carry = min(carry, 4000.0)
        rest = []
        for dl, ready, cost, fn in pending:
            if ready > s:
                rest.append((dl, ready, cost, fn))
                continue
            if dl <= s or carry >= cost:
                place(s, fn)
                carry -= cost
            else:
                rest.append((dl, ready, cost, fn))
        pending = rest
    for dl, ready, cost, fn in pending:
        tail.append(fn)

    # --- emit -------------------------------------------------------------
    # prefix: ACT does the psum->sbuf rope copies (it is idle pre-exp)
    qk_unit(1, 0, 0, 0, act_copy=True)
    qk_unit(0, 0, 0, 0, act_copy=True)
    qk_unit(1, 0, 0, 1, act_copy=True)
    qk_unit(0, 0, 0, 1, act_copy=True)
    for s in range(NSLOT):
        b, kk = s // NKC, s % NKC
        pre, post = [], []
        for fn in slots[s]:
            (post if getattr(fn, "_post", False) else pre).append(fn)
        for fn in pre:
            fn()
        mm1_exp(b, kk)
        for fn in post:
            fn()
    in_tail_flag[0] = True
    for fn in tail:
        fn()
    # PE-transpose aT(1,3) [q, v] -> aTT [v, q] directly from SBUF (no DRAM
    # roundtrip on the critical tail path)
    aT_l = norm_store.last_aT
    for qs in range(QS):
        tp = psum_s.tile([P, P], BF16, name="ps", tag="ps")
        nc.tensor.transpose(
            tp[:],
            aT_l[:, 2 * qs:2 * qs + 2, :].rearrange("p g d -> p (g d)"),
            ident_sb[:],
        )
        if qs % 2 == 0:
            nc.scalar.activation(
                aTT[:, 1, 3 * NQ_BLK + qs * P:3 * NQ_BLK + (qs + 1) * P], tp[:], AF.Copy)
        else:
            nc.vector.tensor_copy(
                aTT[:, 1, 3 * NQ_BLK + qs * P:3 * NQ_BLK + (qs + 1) * P], tp[:])
    # final output-projection block for j=3 (uses the freed MM1 psum pool)
    for u in range(NOB * QS):
        phase3_unit(3 * QS + u // NOB, u % NOB)
    if debug:
        dbg_qt = nc.dram_tensor("dbg_qt", [P, NQ_BLK], BF16, kind="ExternalOutput").ap()
        dbg_kt = nc.dram_tensor("dbg_kt", [P, NQ_BLK], BF16, kind="ExternalOutput").ap()
        dbg_vt = nc.dram_tensor("dbg_vt", [P, NKC, HPC, D], BF16, kind="ExternalOutput").ap()
        dbg_aTT = nc.dram_tensor("dbg_aTT", [P, VF // P, N], BF16, kind="ExternalOutput").ap()
        dbg_es = nc.dram_tensor("dbg_es", [P, 2, NQ_BLK], BF16, kind="ExternalOutput").ap()
        for dst, t in ((dbg_qt, qt[0][0]), (dbg_kt, kt[0][0]), (dbg_vt, vt), (dbg_aTT, aTT)):
            nc.sync.dma_start(dst, t[:])
        nc.sync.dma_start(dbg_es, debug_es[0][:])


def _split_perm(D):
    return np.concatenate([np.arange(0, D, 2), np.arange(1, D, 2)])


def _prep_core_inputs(x, freqs_cis, w_qkv, w_proj, b, heads):
    perm = _split_perm(D)
    qrows, krows = [], []
    for h in heads:
        qrows.append(w_qkv[h * D:(h + 1) * D][perm])
        krows.append(w_qkv[C + h * D:C + (h + 1) * D][perm])
    vrows = [w_qkv[2 * C + h * D:2 * C + (h + 1) * D] for h in heads]
    # per-head-pair column groups: [K_i0, Q_i0, K_i1, Q_i1] so each
    # 128-col DMA slice is contiguous in DRAM
    wqk = np.concatenate(
        [krows[0], krows[1], qrows[0], qrows[1], krows[2], krows[3], qrows[2], qrows[3]],
        axis=0)
    wv = np.concatenate(vrows, axis=0)
    hcols = np.concatenate([np.arange(h * D, (h + 1) * D) for h in heads])
    import ml_dtypes
    bf16 = ml_dtypes.bfloat16
    return {
        "ident": np.eye(128, dtype=np.float32).astype(bf16),
        "xT": np.ascontiguousarray(x[b].T.reshape(C, 8, N // 8).transpose(1, 0, 2)).astype(bf16),
        "wqkT": np.ascontiguousarray(wqk.T.reshape(C, 4, 128).transpose(1, 0, 2)).astype(bf16),
        "wvT": np.ascontiguousarray(wv.T).astype(bf16),
        "wpT": np.ascontiguousarray(w_proj[:, hcols].T).astype(bf16),
        "cosT": np.ascontiguousarray(freqs_cis[:, :, 0].T).astype(bf16),
        "sinT": np.ascontiguousarray(
            np.concatenate([freqs_cis[:, :, 1].T, -freqs_cis[:, :, 1].T], axis=0)
        ).astype(bf16),
    }


_CACHE = {}


def _get_compiled():
    if "nc" not in _CACHE:
        nc = bacc.Bacc("TRN2", target_bir_lowering=False, debug=False)
        with tile.TileContext(nc) as tc:
            with ExitStack() as ctx:
                build_attn_kernel(nc, tc, ctx, N=N, C=C, HPC=HPC, D=D, NQ_BLK=512)
        nc.compile()
        _CACHE["nc"] = nc
    return _CACHE["nc"]


def make_in_maps(x, freqs_cis, w_qkv, w_proj):
    x = np.asarray(x, dtype=np.float32)
    freqs_cis = np.asarray(freqs_cis, dtype=np.float32)
    w_qkv = np.asarray(w_qkv, dtype=np.float32)
    w_proj = np.asarray(w_proj, dtype=np.float32)
    in_maps = []
    for c in range(N_CORES):
        b = c // CORES_PER_BATCH
        hg = c % CORES_PER_BATCH
        heads = list(range(hg * HPC, (hg + 1) * HPC))
        in_maps.append(_prep_core_inputs(x, freqs_cis, w_qkv, w_proj, b, heads))
    return in_maps


def gather_output(results, b_proj):
    out = np.zeros((B, N, C), dtype=np.float32)
    for c in range(N_CORES):
        out[c // CORES_PER_BATCH] += results[c]["y"]
    out += np.asarray(b_proj, dtype=np.float32)[None, None, :]
    return out


def kernel(x, freqs_cis, w_qkv, w_proj, b_proj):
    nc = _get_compiled()
    in_maps = make_in_maps(x, freqs_cis, w_qkv, w_proj)
    res = run_bass_kernel_spmd(nc, in_maps, core_ids=list(range(N_CORES)))
    return gather_output(res.results, b_proj)


# revision 3
# speedup vs baseline: 20.2782x; 20.2782x over previous
"""Trainium2 Bass kernel for nn_Attention_39015482916872 (v3).

Multi-head attention (B=2, N=2048, C=1024, H=16, D=64) with RoPE,
tensor-parallel over (batch, heads) across 8 NeuronCores: core c handles
batch c//4 and heads 4*(c%4)..4*(c%4)+3. Host sums the 4 partial
projections per batch (Megatron column-parallel w_proj) and adds b_proj.

v3 design (cost-model-driven):
 - x is cast to bf16 AND transposed to [C, N] on the host; device does
   plain DMA loads into SBUF.
 - The kernel is emitted as 128 "exp slots" (one per softmax activation,
   the ACT-engine pacer). Per slot: scheduled PE filler units, the two
   score matmuls (MM1), the exp. PV matmuls (MM2, free size 65->64 via a
   separate denominator accumulator) lag their exp by a per-block LAG,
   bounded by a 32-deep es pool; projection / V / output-projection
   units are placed greedily by deadline into the leftover PE budget.
 - PSUM (8 banks): ps pool 2x2 banks, po_all [128,2,8,64] manual
   double-buffer 2 banks, den_all [128,2,8] 1 bank, pmm_all [128,2,256]
   manual double-buffer 1 bank (all projection units have free size 256).
 - Attention output a^T [q, v] -> DRAM -> XBAR transpose -> [v, q] for
   the output projection.
"""

import sys
from contextlib import ExitStack

import numpy as np

if "/opt/trn_rl_repo" not in sys.path:
    sys.path.insert(0, "/opt/trn_rl_repo")
try:
    import concourse.bass as bass
except ImportError:
    sys.path.insert(0, "/root/.axon_site/_ro/trn_rl_repo")
    import concourse.bass as bass
import concourse.tile as tile
from concourse import bacc, mybir
from concourse.bass_utils import run_bass_kernel_spmd

F32 = mybir.dt.float32
BF16 = mybir.dt.bfloat16
AF = mybir.ActivationFunctionType

B, N, C, H, D = 2, 2048, 1024, 16, 64
SCHED = {
    "LAG": [24, 23, 22, 21, 20, 19, 18, 17],
    "QDL": {(0, 1): 13, (0, 2): 26, (0, 3): 42, (1, 0): 57,
            (1, 1): 73, (1, 2): 89, (1, 3): 105},
    "KDL1": 60,
    "VDL_OFF": -2,
}
N_CORES = 8
CORES_PER_BATCH = N_CORES // B          # 4
HPC = H // CORES_PER_BATCH              # 4 heads per core


def build_attn_kernel(nc, tc, ctx, N=2048, C=1024, HPC=4, D=64, NQ_BLK=512, scale=None, debug=False):
    P = 128
    KC = C // P                          # 8 contraction chunks
    VF = HPC * D                         # 256
    NB = N // NQ_BLK                     # 4 q blocks
    NKC = N // P                         # 16 key chunks
    NCH = HPC // 2                       # 2 head pairs
    QS = NQ_BLK // P                     # 4 q sub-blocks per q block
    NBLK = NCH * NB                      # 8 attention blocks
    NSLOT = NBLK * NKC                   # 128 exp slots
    if scale is None:
        scale = D ** -0.5

    xT = nc.dram_tensor("xT", [8, C, N // 8], BF16, kind="ExternalInput").ap()
    wqkT = nc.dram_tensor("wqkT", [4, C, 2 * HPC * D // 4], BF16, kind="ExternalInput").ap()
    wvT = nc.dram_tensor("wvT", [C, VF], BF16, kind="ExternalInput").ap()
    wpT = nc.dram_tensor("wpT", [VF, C], BF16, kind="ExternalInput").ap()
    cosT = nc.dram_tensor("cosT", [D // 2, N], BF16, kind="ExternalInput").ap()
    sinT = nc.dram_tensor("sinT", [D, N], BF16, kind="ExternalInput").ap()
    ident = nc.dram_tensor("ident", [P, P], BF16, kind="ExternalInput").ap()
    y = nc.dram_tensor("y", [N, C], F32, kind="ExternalOutput").ap()
    adr = nc.dram_tensor("adr_internal", [VF // P, N, P], BF16,
                         kind=("ExternalOutput" if debug else "Internal")).ap()

    persist = ctx.enter_context(tc.tile_pool(name="persist", bufs=1))
    psum_s = ctx.enter_context(tc.tile_pool(name="psum_s", bufs=2, space="PSUM"))
    psum_fix = ctx.enter_context(tc.tile_pool(name="psum_fix", bufs=1, space="PSUM"))
    rope_tmp = ctx.enter_context(tc.tile_pool(name="rope_tmp", bufs=6))
    exp_pool = ctx.enter_context(tc.tile_pool(name="exp_pool", bufs=(36 if debug else 40)))
    norm_pool = ctx.enter_context(tc.tile_pool(name="norm_pool", bufs=4))
    y_pool = ctx.enter_context(tc.tile_pool(name="y_pool", bufs=3))

    xT_sb = persist.tile([P, KC, N], BF16, tag="xTsb")
    wqkT_sb = persist.tile([P, KC, 2 * HPC * D], BF16, tag="wqk")
    wvT_sb = persist.tile([P, KC, VF], BF16, tag="wv")
    wpT_sb = persist.tile([P, VF // P, C], BF16, tag="wp")
    cos_sb = persist.tile([P, N], BF16, tag="cos")
    sin_sb = persist.tile([P, N], BF16, tag="sin")
    ones_sb = persist.tile([P, 1], BF16, tag="ones")
    ident_sb = persist.tile([P, P], BF16, tag="ident")
    zeros_sb = persist.tile([P, 512], BF16, tag="zeros")
    qt = [[persist.tile([P, NQ_BLK], BF16, name=f"qt{i}_{j}", tag=f"qt{i}_{j}") for j in range(NB)] for i in range(NCH)]
    kt = [[persist.tile([P, NQ_BLK], BF16, name=f"kt{i}_{j}", tag=f"kt{i}_{j}") for j in range(NB)] for i in range(NCH)]
    vt = persist.tile([P, NKC, HPC, D], BF16, tag="vt")
    aTT = persist.tile([P, VF // P, N], BF16, tag="aTT")

    # PSUM plan: a matmul with start=True zeroes its whole 2KB region, so
    # each open accumulation bank gets ONE dummy start per block and all
    # real matmuls accumulate with start=False. po and den are single
    # banks (blocks serialize through them via the norm WAR dep); pmm is
    # a 4-slice ring for the atomic projection units.
    po_all = psum_fix.tile([P, 2 * QS, D], F32, tag="po_all")
    den_all = psum_fix.tile([P, 2 * QS], F32, tag="den_all")
    pmm_all = psum_fix.tile([P, 4, 256], F32, tag="pmm_all")

    # preload the exp activation table while DMAs stream in
    warm = persist.tile([1, 8], F32, tag="actwarm")
    nc.vector.memset(warm[:], 0.0)
    nc.scalar.activation(warm[:], warm[:], AF.Exp, scale=1.0)
    nc.vector.memset(ones_sb[:], 1.0)
    nc.vector.memset(zeros_sb[:], 0.0)

    # --- startup DMAs -----------------------------------------------------
    # ALL on the SP queue: DMA transfers serialize on the engine pool, so
    # queue emission order is transfer priority.
    def wqk_g(g):
        return wqkT[g].rearrange("(kc p) f -> p kc f", p=P)

    def x_chunk(c):
        return xT[c].rearrange("(kc p) n -> p kc n", p=P)

    nc.sync.dma_start(wqkT_sb[:, :, 0:P], wqk_g(0))           # K i0
    nc.sync.dma_start(xT_sb[:, :, 0:256], x_chunk(0))
    nc.sync.dma_start(wqkT_sb[:, :, P:2 * P], wqk_g(1))       # Q i0
    for g in range(P // (D // 2)):
        nc.sync.dma_start(cos_sb[g * (D // 2):(g + 1) * (D // 2), :], cosT[:, :])
    for g in range(P // D):
        nc.sync.dma_start(sin_sb[g * D:(g + 1) * D, :], sinT[:, :])
    nc.sync.dma_start(xT_sb[:, :, 256:512], x_chunk(1))
    nc.sync.dma_start(xT_sb[:, :, 512:768], x_chunk(2))
    nc.sync.dma_start(xT_sb[:, :, 768:1024], x_chunk(3))
    nc.sync.dma_start(wqkT_sb[:, :, 2 * P:3 * P], wqk_g(2))   # K i1
    nc.sync.dma_start(wqkT_sb[:, :, 3 * P:4 * P], wqk_g(3))   # Q i1
    nc.sync.dma_start(wvT_sb[:], wvT.rearrange("(kc p) f -> p kc f", p=P))
    nc.sync.dma_start(xT_sb[:, :, 1024:1280], x_chunk(4))
    nc.sync.dma_start(xT_sb[:, :, 1280:1536], x_chunk(5))
    nc.sync.dma_start(xT_sb[:, :, 1536:1792], x_chunk(6))
    nc.sync.dma_start(xT_sb[:, :, 1792:2048], x_chunk(7))
    nc.sync.dma_start(wpT_sb[:], wpT.rearrange("(vc p) f -> p vc f", p=P))
    nc.sync.dma_start(ident_sb[:], ident)

    def xs(k, n0, w):
        return xT_sb[:, k, n0:n0 + w]

    # --- projection / filler units (each uses one pmm_all slice) ---------
    pmm_ctr = [0]

    def pmm_slice():
        s = pmm_ctr[0] % 4
        pmm_ctr[0] += 1
        return pmm_all[:, s, :]

    def v_chunk(kk):
        pv = pmm_slice()
        for k in range(KC):
            nc.tensor.matmul(
                pv, lhsT=xs(k, kk * P, P), rhs=wvT_sb[:, k, :],
                start=(k == 0), stop=(k == KC - 1),
            )
        nc.vector.tensor_copy(vt[:, kk, :, :], pv.rearrange("p (h d) -> p h d", d=D))

    def rope_half(psum_c, dst, j, half, act_copy=False):
        nb = j * NQ_BLK + half * 256
        cs = cos_sb[:, nb:nb + 256]
        sn = sin_sb[:, nb:nb + 256]
        raw = rope_tmp.tile([P, 256], BF16, tag="raw")
        if act_copy:
            nc.scalar.activation(raw[:], psum_c, AF.Copy)
        else:
            nc.vector.tensor_copy(raw[:], psum_c)
        tA = rope_tmp.tile([P, 256], BF16, tag="tA")
        tB = rope_tmp.tile([P, 256], BF16, tag="tB")
        nc.vector.tensor_mul(tA[:], raw[:], cs)
        # swapped sin product: out rows swap r<->i; +/- folded into sin table
        for g in range(2):
            b0 = 64 * g
            nc.vector.tensor_mul(tB[b0:b0 + 32, :], raw[b0 + 32:b0 + 64, :], sn[b0 + 32:b0 + 64, :])
            nc.vector.tensor_mul(tB[b0 + 32:b0 + 64, :], raw[b0:b0 + 32, :], sn[b0:b0 + 32, :])
        nc.vector.tensor_add(dst[:, nb - j * NQ_BLK:nb - j * NQ_BLK + 256], tA[:], tB[:])

    def qk_unit(qk, i, j, half, act_copy=False):
        """n-half (256 cols) of a Q/K projection block, incl. RoPE."""
        fbase = i * 2 * P + (1 - qk) * P
        pqk = pmm_slice()
        n0 = j * NQ_BLK + half * 256
        for k in range(KC):
            nc.tensor.matmul(
                pqk, lhsT=wqkT_sb[:, k, fbase:fbase + P], rhs=xs(k, n0, 256),
                start=(k == 0), stop=(k == KC - 1),
            )
        dst = (qt if qk == 0 else kt)[i][j]
        rope_half(pqk, dst, j, half, act_copy=act_copy)

    # --- phase 3: output projection --------------------------------------
    def transpose_a(j, vc):
        nc.sync.dma_start_transpose(
            aTT[:, vc, j * NQ_BLK:(j + 1) * NQ_BLK],
            adr[vc, j * NQ_BLK:(j + 1) * NQ_BLK, :],
        )

    OB = 256
    NOB = C // OB
    yt_tiles = {}
    p3_done = set()
    p3_ctr = [0]

    in_tail_flag = [False]

    def tail_py(u):
        # post-exp psum ring: the 4 pmm slices + the freed MM1 score pool
        if u % 3 == 0:
            py_t = psum_s.tile([P, 2, NQ_BLK], F32, name="ps", tag="ps")
            return py_t[:, 0, 0:OB]
        return pmm_slice()

    def phase3_unit(jj, ob):
        u = p3_ctr[0]
        p3_ctr[0] += 1
        in_tail = in_tail_flag[0]
        if (jj, ob) in p3_done:
            return  # already covered by a widened tail unit
        if in_tail and ob % 2 == 0:
            # tail: 512-wide units in the freed MM1 psum pool (halves the
            # psum-copy count; this unit covers obs {ob, ob+1})
            py_t = psum_s.tile([P, 2, NQ_BLK], F32, name="ps", tag="ps")
            py = py_t[:, 0, :]
            W = 2 * OB
            p3_done.add((jj, ob + 1))
        else:
            py = pmm_slice()
            W = OB
        for vc in range(VF // P):
            nc.tensor.matmul(
                py, lhsT=aTT[:, vc, jj * P:(jj + 1) * P],
                rhs=wpT_sb[:, vc, ob * OB:ob * OB + W],
                start=(vc == 0), stop=(vc == VF // P - 1),
            )
        if jj not in yt_tiles:
            yt_tiles[jj] = y_pool.tile([P, C], F32, name="yt", tag="yt")
        yt = yt_tiles[jj]
        # GPSIMD cannot read PSUM: DVE in-stream; rotate DVE/ACT in the
        # tail (ACT is idle once the exps are done)
        if in_tail and u % 2 == 0:
            nc.scalar.activation(yt[:, ob * OB:ob * OB + W], py, AF.Copy)
        else:
            nc.vector.tensor_copy(yt[:, ob * OB:ob * OB + W], py)
        if ob + W // OB - 1 == NOB - 1:
            nc.sync.dma_start(y[jj * P:(jj + 1) * P, :], yt[:])
            del yt_tiles[jj]

    # --- phase 2 pieces ----------------------------------------------------
    es_tiles = {}
    debug_es = []

    def mm1_exp(b, kk):
        i, j = b // NB, b % NB
        h0 = 2 * i
        ps = psum_s.tile([P, 2, NQ_BLK], F32, tag="ps")
        kb, kc0 = divmod(kk * P, NQ_BLK)
        for g in range(2):
            hb = 64 * g
            nc.tensor.matmul(
                ps[:, g, :],
                lhsT=kt[i][kb][hb:hb + 64, kc0:kc0 + P],
                rhs=qt[i][j][hb:hb + 64, :],
                start=True, stop=True,
            )
        es = exp_pool.tile([P, 2, NQ_BLK], BF16, tag="es")
        nc.scalar.activation(es[:], ps[:], AF.Exp, scale=float(scale))
        es_tiles[(b, kk)] = es
        if (b, kk) == (0, 0) and not debug_es:
            debug_es.append(es)

    def mm2_batch(b, kk):
        i = b // NB
        h0, h1 = 2 * i, 2 * i + 1
        es = es_tiles.pop((b, kk))
        if kk == 0:
            # open the po and den banks: one start=True zeroing matmul each
            nc.tensor.matmul(po_all[:, :, :], lhsT=ident_sb[:], rhs=zeros_sb[:],
                             start=True, stop=False)
            nc.tensor.matmul(den_all[:, :], lhsT=ident_sb[:], rhs=zeros_sb[:, 0:2 * QS],
                             start=True, stop=False)
        last = (kk == NKC - 1)
        for qs in range(QS):
            for g, h in ((0, h0), (1, h1)):
                idx = qs * 2 + g
                lhsT = es[:, g, qs * P:(qs + 1) * P]
                nc.tensor.matmul(
                    po_all[:, idx, :], lhsT=lhsT, rhs=vt[:, kk, h, :],
                    start=False, stop=(last and idx == 2 * QS - 1),
                )
                nc.tensor.matmul(
                    den_all[:, idx:idx + 1], lhsT=lhsT, rhs=ones_sb[:],
                    start=False, stop=(last and idx == 2 * QS - 1),
                )

    def mm2_idx_unit(b, idx):
        i = b // NB
        qs, g = idx // 2, idx % 2
        h = 2 * i + g
        last = 2 * QS - 1
        for kk in range(NKC):
            lhsT = es_tiles[(b, kk)][:, g, qs * P:(qs + 1) * P]
            nc.tensor.matmul(
                po_all[:, idx, :], lhsT=lhsT, rhs=vt[:, kk, h, :],
                start=(kk == 0), stop=(kk == NKC - 1),
            )
        for kk in range(NKC):
            lhsT = es_tiles[(b, kk)][:, g, qs * P:(qs + 1) * P]
            nc.tensor.matmul(
                den_all[:, idx:idx + 1], lhsT=lhsT, rhs=ones_sb[:],
                start=(kk == 0), stop=(kk == NKC - 1),
            )
        if idx == last:
            for kk in range(NKC):
                es_tiles.pop((b, kk))

    def norm_store(b):
        i, j = b // NB, b % NB
        if debug and b == 0:
            dbg_den = nc.dram_tensor("dbg_den", [P, 2 * QS], F32, kind="ExternalOutput").ap()
            dbg_po = nc.dram_tensor("dbg_po", [P, 2 * QS, D], F32, kind="ExternalOutput").ap()
            dent = persist.tile([P, 2 * QS], F32, tag="dbgden")
            pot = persist.tile([P, 2 * QS, D], F32, tag="dbgpo")
            nc.vector.tensor_copy(dent[:], den_all[:, :])
            nc.vector.tensor_copy(pot[:], po_all[:, :, :])
            nc.sync.dma_start(dbg_den, dent[:])
            nc.sync.dma_start(dbg_po, pot[:])
        recip = norm_pool.tile([P, 2 * QS, 1], F32, tag="recip")
        nc.vector.reciprocal(recip[:], den_all[:, :].rearrange("p (q o) -> p q o", o=1))
        aT = norm_pool.tile([P, 2 * QS, D], BF16, tag="aT")
        for idx in range(2 * QS):
            nc.vector.tensor_scalar_mul(
                aT[:, idx, :], po_all[:, idx, :], recip[:, idx, :])
        if i == 1 and j == NB - 1:
            norm_store.last_aT = aT
            return
        dst = adr[i, j * NQ_BLK:(j + 1) * NQ_BLK, :].rearrange(
            "(qs p) (g d) -> p qs g d", p=P, d=D
        )
        nc.sync.dma_start(dst, aT[:].rearrange("p (qs g) d -> p qs g d", g=2))
        transpose_a(j, i)

    # --- static slot schedule ---------------------------------------------
    MM1_NS, MM2_NS = 426, 250
    slots = [[] for _ in range(NSLOT)]
    tail = []

    def place(s, fn):
        (slots[s] if s < NSLOT else tail).append(fn)

    # MM2 trains + norms through the single po/den banks
    # (LAG[b+1] >= LAG[b]-1 keeps the bank serialization legal).
    LAG = SCHED["LAG"]
    mm2_at = {}
    for b in range(NBLK):
        for kk in range(NKC):
            s = 16 * b + kk + LAG[b]
            mm2_at.setdefault(s, []).append((b, kk))
    for s, lst in mm2_at.items():
        for b, kk in lst:
            place(s, (lambda b=b, kk=kk: mm2_batch(b, kk)))
            if kk == NKC - 1:
                place(s, (lambda b=b: norm_store(b)))

    # greedy filler placement: earliest-deadline-first into accumulated
    # PE-idle budget (carry); deadline-due units force regardless of carry.
    units = []   # (deadline, ready, cost, fn)
    kdl = {1: (0, 1), 2: (2, 5), 3: (4, 8)}   # K(0,j): (ready, dl_h0)
    for j, (ready, dl) in kdl.items():
        for half in range(2):
            units.append((dl + half, ready, 853,
                          (lambda j=j, h=half: qk_unit(1, 0, j, h))))
    qdl = SCHED["QDL"]
    for (i, j), dl in qdl.items():
        for half in range(2):
            units.append((dl + half, 1 if j > 0 or i > 0 else 0, 853,
                          (lambda i=i, j=j, h=half: qk_unit(0, i, j, h))))
    for kb in range(NB):
        for half in range(2):
            units.append((SCHED["KDL1"] + 4 * kb + half, 1, 853,
                          (lambda kb=kb, h=half: qk_unit(1, 1, kb, h))))
    for kk in range(NKC):
        ready = 1 if kk < 8 else (3 if kk < 12 else 5)
        units.append((kk + LAG[0] + SCHED["VDL_OFF"], ready, 853,
                      (lambda kk=kk: v_chunk(kk))))
    for j in range(3):
        ready = 16 * (4 + j) + 15 + LAG[4 + j] + 2
        for u in range(NOB * QS):
            # staggered deadlines force ~1 unit/slot right after readiness
            units.append((min(ready + 2 + u, NSLOT - 1), ready, 240,
                          (lambda j=j, u=u: phase3_unit(j * QS + u // NOB, u % NOB))))

    units.sort(key=lambda t: (t[0], t[1]))
    pending = list(units)
    carry = 0.0
    for s in range(NSLOT):
        carry += 1038 - MM1_NS - MM2_NS * len(mm2_at.get(s, []))
        carry = min(carry, 4000.0)
        rest = []
        for dl, ready, cost, fn in pending:
            if ready > s:
                rest.append((dl, ready, cost, fn))
                continue
            if dl <= s or carry >= cost:
                place(s, fn)
                carry -= cost
            else:
                rest.append((dl, ready, cost, fn))
        pending = rest
    for dl, ready, cost, fn in pending:
        tail.append(fn)

    # --- emit -------------------------------------------------------------
    # prefix: ACT does the psum->sbuf rope copies (it is idle pre-exp)
    qk_unit(1, 0, 0, 0, act_copy=True)
    qk_unit(0, 0, 0, 0, act_copy=True)
    qk_unit(1, 0, 0, 1, act_copy=True)
    qk_unit(0, 0, 0, 1, act_copy=True)
    for s in range(NSLOT):
        b, kk = s // NKC, s % NKC
        pre, post = [], []
        for fn in slots[s]:
            (post if getattr(fn, "_post", False) else pre).append(fn)
        for fn in pre:
            fn()
        mm1_exp(b, kk)
        for fn in post:
            fn()
    in_tail_flag[0] = True
    for fn in tail:
        fn()
    # PE-transpose aT(1,3) [q, v] -> aTT [v, q] directly from SBUF (no DRAM
    # roundtrip on the critical tail path)
    aT_l = norm_store.last_aT
    for qs in range(QS):
        tp = psum_s.tile([P, P], BF16, name="ps", tag="ps")
        nc.tensor.transpose(
            tp[:],
            aT_l[:, 2 * qs:2 * qs + 2, :].rearrange("p g d -> p (g d)"),
            ident_sb[:],
        )
        if qs % 2 == 0:
            nc.scalar.activation(
                aTT[:, 1, 3 * NQ_BLK + qs * P:3 * NQ_BLK + (qs + 1) * P], tp[:], AF.Copy)
        else:
            nc.vector.tensor_copy(
                aTT[:, 1, 3 * NQ_BLK + qs * P:3 * NQ_BLK + (qs + 1) * P], tp[:])
    # final output-projection block for j=3 (uses the freed MM1 psum pool)
    for u in range(NOB * QS):
        phase3_unit(3 * QS + u // NOB, u % NOB)
    if debug:
        dbg_qt = nc.dram_tensor("dbg_qt", [P, NQ_BLK], BF16, kind="ExternalOutput").ap()
        dbg_kt = nc.dram_tensor("dbg_kt", [P, NQ_BLK], BF16, kind="ExternalOutput").ap()
        dbg_vt = nc.dram_tensor("dbg_vt", [P, NKC, HPC, D], BF16, kind="ExternalOutput").ap()
        dbg_aTT = nc.dram_tensor("dbg_aTT", [P, VF // P, N], BF16, kind="ExternalOutput").ap()
        dbg_es = nc.dram_tensor("dbg_es", [P, 2, NQ_BLK], BF16, kind="ExternalOutput").ap()
        for dst, t in ((dbg_qt, qt[0][0]), (dbg_kt, kt[0][0]), (dbg_vt, vt), (dbg_aTT, aTT)):
            nc.sync.dma_start(dst, t[:])
        nc.sync.dma_start(dbg_es, debug_es[0][:])


def _split_perm(D):
    return np.concatenate([np.arange(0, D, 2), np.arange(1, D, 2)])


def _prep_core_inputs(x, freqs_cis, w_qkv, w_proj, b, heads):
    perm = _split_perm(D)
    qrows, krows = [], []
    for h in heads:
        qrows.append(w_qkv[h * D:(h + 1) * D][perm])
        krows.append(w_qkv[C + h * D:C + (h + 1) * D][perm])
    vrows = [w_qkv[2 * C + h * D:2 * C + (h + 1) * D] for h in heads]
    # per-head-pair column groups: [K_i0, Q_i0, K_i1, Q_i1] so each
    # 128-col DMA slice is contiguous in DRAM
    wqk = np.concatenate(
        [krows[0], krows[1], qrows[0], qrows[1], krows[2], krows[3], qrows[2], qrows[3]],
        axis=0)
    wv = np.concatenate(vrows, axis=0)
    hcols = np.concatenate([np.arange(h * D, (h + 1) * D) for h in heads])
    import ml_dtypes
    bf16 = ml_dtypes.bfloat16
    return {
        "ident": np.eye(128, dtype=np.float32).astype(bf16),
        "xT": np.ascontiguousarray(x[b].T.reshape(C, 8, N // 8).transpose(1, 0, 2)).astype(bf16),
        "wqkT": np.ascontiguousarray(wqk.T.reshape(C, 4, 128).transpose(1, 0, 2)).astype(bf16),
        "wvT": np.ascontiguousarray(wv.T).astype(bf16),
        "wpT": np.ascontiguousarray(w_proj[:, hcols].T).astype(bf16),
        "cosT": np.ascontiguousarray(freqs_cis[:, :, 0].T).astype(bf16),
        "sinT": np.ascontiguousarray(
            np.concatenate([freqs_cis[:, :, 1].T, -freqs_cis[:, :, 1].T], axis=0)
        ).astype(bf16),
    }


_CACHE = {}


def _get_compiled():
    if "nc" not in _CACHE:
        nc = bacc.Bacc("TRN2", target_bir_lowering=False, debug=False)
        with tile.TileContext(nc) as tc:
            with ExitStack() as ctx:
                build_attn_kernel(nc, tc, ctx, N=N, C=C, HPC=HPC, D=D, NQ_BLK=512)
        nc.compile()
        _CACHE["nc"] = nc
    return _CACHE["nc"]


def make_in_maps(x, freqs_cis, w_qkv, w_proj):
    x = np.asarray(x, dtype=np.float32)
    freqs_cis = np.asarray(freqs_cis, dtype=np.float32)
    w_qkv = np.asarray(w_qkv, dtype=np.float32)
    w_proj = np.asarray(w_proj, dtype=np.float32)
    in_maps = []
    for c in range(N_CORES):
        b = c // CORES_PER_BATCH
        hg = c % CORES_PER_BATCH
        heads = list(range(hg * HPC, (hg + 1) * HPC))
        in_maps.append(_prep_core_inputs(x, freqs_cis, w_qkv, w_proj, b, heads))
    return in_maps


def gather_output(results, b_proj):
    out = np.zeros((B, N, C), dtype=np.float32)
    for c in range(N_CORES):
        out[c // CORES_PER_BATCH] += results[c]["y"]
    out += np.asarray(b_proj, dtype=np.float32)[None, None, :]
    return out


def kernel(x, freqs_cis, w_qkv, w_proj, b_proj):
    nc = _get_compiled()
    in_maps = make_in_maps(x, freqs_cis, w_qkv, w_proj)
    res = run_bass_kernel_spmd(nc, in_maps, core_ids=list(range(N_CORES)))
    return gather_output(res.results, b_proj)
